# revision 31
# baseline (speedup 1.0000x reference)
"""3-layer GAT + linear head on 8 TRN2 NeuronCores (Bass/Tile).

Sharding:
  - Nodes split into 8 contiguous blocks; core k owns block k and every edge
    whose destination lies in its block.
  - Layer 1 has NO collective: x is replicated (host-staged), so every core
    computes the FULL [N, 512] h1 bf16 table locally (1024B rows).  The
    per-edge attention logits a_s1[src]+a_d1[dst] depend only on x, so they
    are precomputed on the host and staged per edge slot (aeE).
  - Layers 2/3: each core computes h|a_s (bf16, 256B rows) for its own
    nodes, AllGathers the [N, 128] bf16 table, then aggregates its edges.
    a_d[dst] is tile-constant: the local [P, NT] a_d matrix is PE-transposed
    once per layer, round-tripped through DRAM as a flat row, and
    partition-broadcast once per tile (adR).
  - Edge phase per 128-destination tile, per chunk of 128 edge slots
    (slot e=ch*128+p -> partition p, chunk ch; pad slots fetch row 0):
      * dma_gather of bf16 source rows from the local table copy,
      * layer 1: w = exp(leakyrelu(aeE)) per head; batched one-hot build
        S01[e,ch,d] (bf16) on DVE; out += S01.T @ (w*G); s += S01.T @ w.
      * layers 2/3 fold the weight into the one-hot, all batched per group
        in bf16: W01[e,ch,d] = (dst_loc==d) * exp(leakyrelu(a_s + adR));
        a ones-column written into the gathered rows folds the softmax
        denominator into the same matmul: [out | s] += W01.T @ [G | 1].
      * epilogue: out/s, +bias, ELU; PE-transpose for the next matmul.
  - int16 gather indices: node table split in two N/2-row halves, each
    dst-tile's edges pre-split by source half (groups A/B).

Self-contained; hardcodes shapes for N=50000, E=800000, D_IN=128, HID=64,
HEADS=8, D_OUT=10.
"""
import os
import numpy as np
import ml_dtypes

import concourse.bass as bass
import concourse.mybir as mybir
import concourse.tile as tile
from concourse import bacc
from concourse.bass_utils import run_bass_kernel_spmd
from concourse.masks import make_identity

_MINI = bool(int(os.environ.get("GAT_MINI", "0")))
if _MINI:
    N = 2048
    E = 16384
else:
    N = 50000
    E = 800000
NCORES = 8
VP = N // NCORES          # nodes per core
P = 128
NT = (VP + P - 1) // P    # dst tiles per core
NTP = NT * P
NTF = (N + P - 1) // P    # full-table tiles
NFP = NTF * P
OV0 = int(N * 0.35)      # B-half table starts here (idx = src - OV0)
OV1 = N - OV0             # A-half table covers rows [0, OV1)
D_IN = 128
HID = 64
HEADS = 8
D_OUT = 10
R1 = 512                  # layer-1 row: h(512) bf16 = 1024B
R2 = 128                  # layer-2/3 row: h(64) | a_s(1) | pad(63) bf16 = 256B

f32 = mybir.dt.float32
bf16 = mybir.dt.bfloat16
i16 = mybir.dt.int16
AT = mybir.AluOpType
AF = mybir.ActivationFunctionType
bfnp = ml_dtypes.bfloat16


def _prep_edges(edge_index):
    src = np.concatenate([np.asarray(edge_index[0]), np.arange(N)]).astype(np.int64)
    dst = np.concatenate([np.asarray(edge_index[1]), np.arange(N)]).astype(np.int64)

    per_core = []
    cnts = [np.ones(NT, np.int64), np.ones(NT, np.int64)]
    for k in range(NCORES):
        m = (dst >= k * VP) & (dst < (k + 1) * VP)
        s_k = src[m]
        dloc = dst[m] - k * VP
        t_k = dloc // P
        w_k = dloc % P
        tiles = []
        for t in range(NT):
            sel = t_k == t
            ss, ww = s_k[sel], w_k[sel]
            n = len(ss)
            # balanced split: src<OV0 must go to A, src>=OV1 to B, the
            # overlap range is flexible -- equalize the two groups
            order = np.argsort(np.where(ss < OV0, 0, np.where(ss >= OV1, 2, 1)),
                               kind="stable")
            ss, ww = ss[order], ww[order]
            c1 = int((ss < OV0).sum())
            c2 = int((ss >= OV1).sum())
            nA = min(max((n + 1) // 2, c1), n - c2)
            groups = [
                (ss[:nA].astype(np.int64), ww[:nA].astype(np.int64)),
                ((ss[nA:] - OV0).astype(np.int64), ww[nA:].astype(np.int64)),
            ]
            for g in range(2):
                cnts[g][t] = max(cnts[g][t],
                                 (len(groups[g][0]) + P - 1) // P)
            tiles.append(groups)
        per_core.append(tiles)

    chs = [int(c.max()) for c in cnts]
    idx_arrs, dst_arrs = [], []
    for g in range(2):
        ch = chs[g]
        ia, da = [], []
        for k in range(NCORES):
            A = np.zeros((NT, P, ch * 8), np.int16)
            D = np.full((NT, P, ch), -1.0, ml_dtypes.bfloat16)
            for t in range(NT):
                li, ww = per_core[k][t][g]
                n = len(li)
                ii = np.arange(n)
                wrap = np.zeros((16, ch * 8), np.int16)
                wrap[ii % 16, ii // 16] = li.astype(np.int16)
                A[t] = np.tile(wrap, (8, 1))
                D[t, ii % P, ii // P] = ww
            ia.append(A)
            da.append(D)
        idx_arrs.append(ia)
        dst_arrs.append(da)
    return chs, cnts, idx_arrs, dst_arrs


def _edge_phase(nc, tc, layer, CHs, cnts, idx_ins, dst_ins, adE_ins, hfull,
                Rrow, heads, adT, iota, ident, brep, hT_d):
    """adE_ins: layer 1 -> per-slot a_s+a_d logit inputs (A, B); else None.
    adT: layers 2/3 -> DRAM [1, NTP] of transposed per-tile a_d rows.
    cnts: per-tile chunk counts (max over cores) for groups A/B."""
    HC = 512 if layer == 1 else HID
    nbuf = 2 if layer == 1 else 3
    with tc.tile_pool(name=f"e{layer}", bufs=nbuf) as ep, \
         tc.tile_pool(name=f"e{layer}w", bufs=6) as wp, \
         tc.tile_pool(name=f"e{layer}s", bufs=4) as sp, \
         tc.tile_pool(name=f"e{layer}r", bufs=1) as rp, \
         tc.tile_pool(name=f"e{layer}p1", bufs=nbuf, space="PSUM") as pp, \
         tc.tile_pool(name=f"e{layer}p2", bufs=nbuf, space="PSUM") as p2:
        if layer != 1:
            adRow = rp.tile([1, NTP], bf16, tag="adRow")
            nc.sync.dma_start(out=adRow[:], in_=adT[:, :])
        for t in range(NT):
            if heads == 8:
                outu = p2.tile([P, HC], f32, space="PSUM", tag="outu")
                ssum = p2.tile([P, heads], f32, space="PSUM", tag="ssum")
            else:
                outu = p2.tile([P, HID + 2], f32, space="PSUM", tag="outu")
            if layer != 1:
                adR = wp.tile([P, P], bf16, tag="adR")
                nc.gpsimd.partition_broadcast(
                    adR[:], adRow[0:1, t * P:(t + 1) * P])
            lastg, lastch = 1, int(cnts[1][t]) - 1
            for g in range(2):
                CH = int(cnts[g][t])
                NIDX = CH * P
                idxt = wp.tile([P, CHs[g] * 8], i16, tag="idx")
                nc.sync.dma_start(out=idxt[:, 0:CH * 8],
                                  in_=idx_ins[g][t, :, 0:CH * 8])
                dstt = wp.tile([P, CHs[g]], bf16, tag="dst")
                nc.sync.dma_start(out=dstt[:, 0:CH],
                                  in_=dst_ins[g][t, :, 0:CH])
                G = ep.tile([P, CHs[g], Rrow], bf16, tag="G")
                half = hfull[0:OV1, :] if g == 0 else hfull[OV0:N, :]
                nc.gpsimd.dma_gather(G[:, 0:CH, :], half, idxt[:, 0:CH * 8],
                                     NIDX, NIDX, Rrow, single_packet=False)
                if layer == 1:
                    aeE = wp.tile([P, CHs[g], 8], bf16, tag="aeE")
                    nc.sync.dma_start(out=aeE[:, 0:CH, :],
                                      in_=adE_ins[g][t, :, 0:CH * 8])
                    est = wp.tile([P, CHs[g], 8], f32, tag="est")
                    efb = wp.tile([P, CHs[g], 8], bf16, tag="efb")
                    ae = aeE[:, 0:CH, :].rearrange("p a b -> p (a b)")
                    ef = est[:, 0:CH, :].rearrange("p a b -> p (a b)")
                    nc.vector.scalar_tensor_tensor(
                        out=ef, in0=ae, scalar=0.2, in1=ae,
                        op0=AT.mult, op1=AT.max)
                    nc.scalar.activation(
                        efb[:, 0:CH, :].rearrange("p a b -> p (a b)"), ef,
                        AF.Exp)
                    s01a = sp.tile([P, CHs[g], P], bf16, tag="s01")
                    nc.vector.tensor_tensor(
                        out=s01a[:, 0:CH, :],
                        in0=iota[:].rearrange("p (o f) -> p o f", o=1)
                            .to_broadcast([P, CH, P]),
                        in1=dstt[:, 0:CH].rearrange("p (a o) -> p a o", o=1)
                            .to_broadcast([P, CH, P]),
                        op=AT.is_equal)
                    for ch in range(CH):
                        gv = G[:, ch, 0:512].rearrange("p (h c) -> p h c", h=8)
                        nc.vector.tensor_tensor(
                            out=gv, in0=gv,
                            in1=efb[:, ch, :].to_broadcast([P, 8, 64]),
                            op=AT.mult)
                        first = (g == 0 and ch == 0)
                        last = (g == lastg and ch == lastch)
                        nc.tensor.matmul(outu[:], lhsT=s01a[:, ch, :],
                                         rhs=G[:, ch, 0:HC],
                                         start=first, stop=last,
                                         skip_group_check=True)
                        nc.tensor.matmul(ssum[:], lhsT=s01a[:, ch, :],
                                         rhs=efb[:, ch, :],
                                         start=first, stop=last,
                                         skip_group_check=True)
                else:
                    nc.vector.memset(G[:, 0:CH, HID + 1:HID + 2], 1.0)
                    arga = wp.tile([P, CHs[g], P], bf16, tag="argt")
                    nc.vector.tensor_tensor(
                        out=arga[:, 0:CH, :],
                        in0=G[:, 0:CH, HID:HID + 1].to_broadcast([P, CH, P]),
                        in1=adR[:].rearrange("p (o f) -> p o f", o=1)
                            .to_broadcast([P, CH, P]),
                        op=AT.add)
                    af_ = arga[:, 0:CH, :].rearrange("p a b -> p (a b)")
                    nc.vector.scalar_tensor_tensor(
                        out=af_, in0=af_, scalar=0.2, in1=af_,
                        op0=AT.mult, op1=AT.max)
                    nc.scalar.activation(af_, af_, AF.Exp)
                    s01a = sp.tile([P, CHs[g], P], bf16, tag="s01")
                    nc.vector.tensor_tensor(
                        out=s01a[:, 0:CH, :],
                        in0=iota[:].rearrange("p (o f) -> p o f", o=1)
                            .to_broadcast([P, CH, P]),
                        in1=dstt[:, 0:CH].rearrange("p (a o) -> p a o", o=1)
                            .to_broadcast([P, CH, P]),
                        op=AT.is_equal)
                    nc.vector.tensor_tensor(
                        out=s01a[:, 0:CH, :], in0=s01a[:, 0:CH, :],
                        in1=arga[:, 0:CH, :], op=AT.mult)
                    for ch in range(CH):
                        first = (g == 0 and ch == 0)
                        last = (g == lastg and ch == lastch)
                        nc.tensor.matmul(outu[:], lhsT=s01a[:, ch, :],
                                         rhs=G[:, ch, 0:HID + 2],
                                         start=first, stop=last,
                                         skip_group_check=True)
            # epilogue: normalize, bias, ELU
            rec = wp.tile([P, heads], f32, tag="rec")
            ho = ep.tile([P, HC], f32, tag="ho")
            if heads == 8:
                nc.vector.reciprocal(rec[:], ssum[:])
                hv = ho[:].rearrange("p (h c) -> p h c", h=8)
                ov = outu[:].rearrange("p (h c) -> p h c", h=8)
                nc.vector.tensor_tensor(
                    out=hv, in0=ov, in1=rec[:].to_broadcast([P, 8, 64]),
                    op=AT.mult)
                nc.vector.tensor_tensor(out=ho[:], in0=ho[:], in1=brep[:],
                                        op=AT.add)
            else:
                nc.vector.reciprocal(rec[:], outu[:, HID + 1:HID + 2])
                nc.vector.scalar_tensor_tensor(
                    out=ho[:], in0=outu[:, 0:HID], scalar=rec[:, 0:1],
                    in1=brep[:], op0=AT.mult, op1=AT.add)
            el = ep.tile([P, HC], f32, tag="el")
            nc.vector.tensor_scalar(out=el[:], in0=ho[:], scalar1=0.0,
                                    scalar2=None, op0=AT.min)
            nc.scalar.activation(el[:], el[:], AF.Exp)
            nc.vector.scalar_tensor_tensor(
                out=ho[:], in0=ho[:], scalar=0.0, in1=el[:],
                op0=AT.max, op1=AT.add)
            nc.scalar.activation(ho[:], ho[:], AF.Copy, bias=-1.0)
            # transpose for next layer's matmul (lhsT layout, bf16)
            if HC == 512:
                for cb in range(4):
                    tp_ps = pp.tile([P, P], f32, space="PSUM", tag="tp")
                    nc.tensor.transpose(out=tp_ps[:],
                                        in_=ho[:, cb * P:(cb + 1) * P],
                                        identity=ident[:])
                    tsb = wp.tile([P, P], bf16, tag="tsb")
                    nc.vector.tensor_copy(tsb[:], tp_ps[:])
                    nc.sync.dma_start(
                        out=hT_d[cb * P:(cb + 1) * P, t * P:(t + 1) * P],
                        in_=tsb[:])
            else:
                tp_ps = pp.tile([P, P], f32, space="PSUM", tag="tp")
                nc.tensor.transpose(out=tp_ps[:HID, :], in_=ho[:],
                                    identity=ident[:])
                tsb = wp.tile([HID, P], bf16, tag="tsb64")
                nc.vector.tensor_copy(tsb[:], tp_ps[:HID, :])
                nc.sync.dma_start(out=hT_d[:, t * P:(t + 1) * P], in_=tsb[:])


PHASE_ORDER = ["m1", "ag1", "e1", "m2", "ag2", "e2", "m3", "ag3", "e3", "full"]


def _build_program(CHa, CHb, cnts):
    stop = os.environ.get("GAT_STOP", "full")
    lvl = PHASE_ORDER.index(stop) + 1
    nc = bacc.Bacc("TRN2", target_bir_lowering=False, debug=False,
                   enable_asserts=False, num_devices=NCORES)

    xTf_in = nc.dram_tensor("xTf", [P, NFP], bf16, kind="ExternalInput")
    idxA_in = nc.dram_tensor("idxA", [NT, P, CHa * 8], i16, kind="ExternalInput")
    idxB_in = nc.dram_tensor("idxB", [NT, P, CHb * 8], i16, kind="ExternalInput")
    dstA_in = nc.dram_tensor("dstA", [NT, P, CHa], bf16, kind="ExternalInput")
    dstB_in = nc.dram_tensor("dstB", [NT, P, CHb], bf16, kind="ExternalInput")
    adEA_in = nc.dram_tensor("adEA", [NT, P, CHa * 8], bf16, kind="ExternalInput")
    adEB_in = nc.dram_tensor("adEB", [NT, P, CHb * 8], bf16, kind="ExternalInput")
    W1T_in = nc.dram_tensor("W1T", [D_IN, 512], bf16, kind="ExternalInput")
    W2T_in = nc.dram_tensor("W2T", [512, HID], bf16, kind="ExternalInput")
    M2d_in = nc.dram_tensor("M2d", [512, 1], bf16, kind="ExternalInput")
    M2s_in = nc.dram_tensor("M2s", [512, 1], bf16, kind="ExternalInput")
    W3T_in = nc.dram_tensor("W3T", [HID, HID], bf16, kind="ExternalInput")
    M3d_in = nc.dram_tensor("M3d", [HID, 1], bf16, kind="ExternalInput")
    M3s_in = nc.dram_tensor("M3s", [HID, 1], bf16, kind="ExternalInput")
    WcT_in = nc.dram_tensor("WcT", [HID, D_OUT], bf16, kind="ExternalInput")
    b1r_in = nc.dram_tensor("b1r", [P, 512], f32, kind="ExternalInput")
    b2r_in = nc.dram_tensor("b2r", [P, HID], f32, kind="ExternalInput")
    b3r_in = nc.dram_tensor("b3r", [P, HID], f32, kind="ExternalInput")
    bcr_in = nc.dram_tensor("bcr", [P, D_OUT], f32, kind="ExternalInput")

    out_d = nc.dram_tensor("out", [NTP, D_OUT], f32, kind="ExternalOutput")

    hcat1_full = nc.dram_tensor("hcat1_full", [NFP, R1], bf16, kind="Internal")
    hcat2_loc = nc.dram_tensor("hcat2_loc", [VP, R2], bf16, kind="Internal")
    hcat2_full = nc.dram_tensor("hcat2_full", [N, R2], bf16, kind="Internal",
                                addr_space="Shared")
    hcat3_loc = nc.dram_tensor("hcat3_loc", [VP, R2], bf16, kind="Internal")
    hcat3_full = nc.dram_tensor("hcat3_full", [N, R2], bf16, kind="Internal",
                                addr_space="Shared")
    ad2T_d = nc.dram_tensor("ad2T_d", [1, NTP], bf16, kind="Internal")
    ad3T_d = nc.dram_tensor("ad3T_d", [1, NTP], bf16, kind="Internal")
    h1T_d = nc.dram_tensor("h1T_d", [512, NTP], bf16, kind="Internal")
    h2T_d = nc.dram_tensor("h2T_d", [HID, NTP], bf16, kind="Internal")
    h3T_d = nc.dram_tensor("h3T_d", [HID, NTP], bf16, kind="Internal")

    def rows_of(t):
        return P if t < NT - 1 else VP - (NT - 1) * P

    rg = [list(range(NCORES))]

    with tile.TileContext(nc) as tc:
        with tc.tile_pool(name="const", bufs=1) as cs:
            ident = cs.tile([P, P], f32)
            make_identity(nc, ident[:])
            iota = cs.tile([P, P], bf16)
            nc.gpsimd.iota(iota[:], pattern=[[1, P]], base=0,
                           channel_multiplier=0,
                           allow_small_or_imprecise_dtypes=True)

            def c_load(name, shape, src, dt=f32):
                tl = cs.tile(shape, dt, tag=name)
                nc.sync.dma_start(out=tl[:], in_=src)
                return tl

            W1T = c_load("W1T", [D_IN, 512], W1T_in[:], bf16)
            W2Tc = cs.tile([P, 4 * HID], bf16)
            M2dc = cs.tile([P, 4], bf16)
            M2sc = cs.tile([P, 4], bf16)
            for cb in range(4):
                nc.sync.dma_start(out=W2Tc[:, cb * HID:(cb + 1) * HID],
                                  in_=W2T_in[cb * P:(cb + 1) * P, :])
                nc.sync.dma_start(out=M2dc[:, cb:cb + 1],
                                  in_=M2d_in[cb * P:(cb + 1) * P, :])
                nc.sync.dma_start(out=M2sc[:, cb:cb + 1],
                                  in_=M2s_in[cb * P:(cb + 1) * P, :])
            W3T = c_load("W3T", [HID, HID], W3T_in[:], bf16)
            M3d = c_load("M3d", [HID, 1], M3d_in[:], bf16)
            M3s = c_load("M3s", [HID, 1], M3s_in[:], bf16)
            WcT = c_load("WcT", [HID, D_OUT], WcT_in[:], bf16)
            b1r = c_load("b1r", [P, 512], b1r_in[:])
            b2r = c_load("b2r", [P, HID], b2r_in[:])
            b3r = c_load("b3r", [P, HID], b3r_in[:])
            bcr = c_load("bcr", [P, D_OUT], bcr_in[:])
            ad2 = cs.tile([P, NT], f32)
            ad3 = cs.tile([P, NT], f32)
            ad2T = cs.tile([NT, P], bf16)
            ad3T = cs.tile([NT, P], bf16)

            # ---- M1: full h1|a_s table (x replicated; no collective) ----
            if lvl >= 1:
             with tc.tile_pool(name="m1", bufs=3) as mp, \
                 tc.tile_pool(name="m1p", bufs=2, space="PSUM") as mpp:
                for t in range(NTF):
                    xt = mp.tile([P, P], bf16, tag="xt")
                    nc.sync.dma_start(out=xt[:],
                                      in_=xTf_in[:, t * P:(t + 1) * P])
                    h_ps = mpp.tile([P, 512], f32, space="PSUM", tag="h")
                    nc.tensor.matmul(h_ps[:], lhsT=xt[:], rhs=W1T[:],
                                     start=True, stop=True)
                    hc = mp.tile([P, R1], bf16, tag="hc")
                    nc.vector.tensor_copy(hc[:], h_ps[:])
                    nc.sync.dma_start(out=hcat1_full[t * P:(t + 1) * P, :],
                                      in_=hc[:])

            if lvl >= 3:
             _edge_phase(nc, tc, 1, (CHa, CHb), cnts, (idxA_in, idxB_in),
                        (dstA_in, dstB_in), (adEA_in, adEB_in),
                        hcat1_full, R1, 8, None,
                        iota, ident, b1r, h1T_d)

            # ---- M2 ----
            if lvl >= 4:
             with tc.tile_pool(name="m2", bufs=3) as mp, \
                 tc.tile_pool(name="m2p", bufs=2, space="PSUM") as mpp:
                for t in range(NT):
                    h2_ps = mpp.tile([P, HID], f32, space="PSUM", tag="h")
                    ad_ps = mpp.tile([P, 1], f32, space="PSUM", tag="ad")
                    as_ps = mpp.tile([P, 1], f32, space="PSUM", tag="as")
                    for cb in range(4):
                        lt = mp.tile([P, P], bf16, tag="lt")
                        nc.sync.dma_start(
                            out=lt[:],
                            in_=h1T_d[cb * P:(cb + 1) * P, t * P:(t + 1) * P])
                        nc.tensor.matmul(
                            h2_ps[:], lhsT=lt[:],
                            rhs=W2Tc[:, cb * HID:(cb + 1) * HID],
                            start=(cb == 0), stop=(cb == 3))
                        nc.tensor.matmul(ad_ps[:], lhsT=lt[:],
                                         rhs=M2dc[:, cb:cb + 1],
                                         start=(cb == 0), stop=(cb == 3))
                        nc.tensor.matmul(as_ps[:], lhsT=lt[:],
                                         rhs=M2sc[:, cb:cb + 1],
                                         start=(cb == 0), stop=(cb == 3))
                    hc = mp.tile([P, R2], bf16, tag="hc")
                    nc.vector.tensor_copy(hc[:, 0:HID], h2_ps[:])
                    nc.scalar.activation(hc[:, HID:HID + 1], as_ps[:],
                                         AF.Copy)
                    nc.vector.memset(hc[:, HID + 1:R2], 0.0)
                    nc.scalar.activation(ad2[:, t:t + 1], ad_ps[:],
                                         AF.Copy)
                    r = rows_of(t)
                    nc.sync.dma_start(out=hcat2_loc[t * P:t * P + r, :],
                                      in_=hc[:r, :])
                tp_ps = mpp.tile([P, P], f32, space="PSUM", tag="adt")
                nc.tensor.transpose(out=tp_ps[:NT, :], in_=ad2[:],
                                    identity=ident[:])
                nc.vector.tensor_copy(ad2T[:], tp_ps[:NT, :])
                nc.sync.dma_start(
                    out=ad2T_d[0, :].rearrange("(a b) -> a b", a=NT),
                    in_=ad2T[:])
            if lvl >= 5:
             nc.gpsimd.collective_compute(
                "AllGather", AT.bypass, replica_groups=rg,
                ins=[hcat2_loc[:]], outs=[hcat2_full[:]])

            if lvl >= 6:
             _edge_phase(nc, tc, 2, (CHa, CHb), cnts, (idxA_in, idxB_in),
                        (dstA_in, dstB_in), None,
                        hcat2_full, R2, 1, ad2T_d,
                        iota, ident, b2r, h2T_d)

            # ---- M3 ----
            if lvl >= 7:
             with tc.tile_pool(name="m3", bufs=3) as mp, \
                 tc.tile_pool(name="m3p", bufs=2, space="PSUM") as mpp:
                for t in range(NT):
                    lt = mp.tile([HID, P], bf16, tag="lt")
                    nc.sync.dma_start(out=lt[:],
                                      in_=h2T_d[:, t * P:(t + 1) * P])
                    h3_ps = mpp.tile([P, HID], f32, space="PSUM", tag="h")
                    nc.tensor.matmul(h3_ps[:], lhsT=lt[:], rhs=W3T[:],
                                     start=True, stop=True)
                    ad_ps = mpp.tile([P, 1], f32, space="PSUM", tag="ad")
                    as_ps = mpp.tile([P, 1], f32, space="PSUM", tag="as")
                    nc.tensor.matmul(ad_ps[:], lhsT=lt[:], rhs=M3d[:],
                                     start=True, stop=True)
                    nc.tensor.matmul(as_ps[:], lhsT=lt[:], rhs=M3s[:],
                                     start=True, stop=True)
                    hc = mp.tile([P, R2], bf16, tag="hc")
                    nc.vector.tensor_copy(hc[:, 0:HID], h3_ps[:])
                    nc.scalar.activation(hc[:, HID:HID + 1], as_ps[:],
                                         AF.Copy)
                    nc.vector.memset(hc[:, HID + 1:R2], 0.0)
                    nc.scalar.activation(ad3[:, t:t + 1], ad_ps[:],
                                         AF.Copy)
                    r = rows_of(t)
                    nc.sync.dma_start(out=hcat3_loc[t * P:t * P + r, :],
                                      in_=hc[:r, :])
                tp_ps = mpp.tile([P, P], f32, space="PSUM", tag="adt")
                nc.tensor.transpose(out=tp_ps[:NT, :], in_=ad3[:],
                                    identity=ident[:])
                nc.vector.tensor_copy(ad3T[:], tp_ps[:NT, :])
                nc.sync.dma_start(
                    out=ad3T_d[0, :].rearrange("(a b) -> a b", a=NT),
                    in_=ad3T[:])
            if lvl >= 8:
             nc.gpsimd.collective_compute(
                "AllGather", AT.bypass, replica_groups=rg,
                ins=[hcat3_loc[:]], outs=[hcat3_full[:]])

            if lvl >= 9:
             _edge_phase(nc, tc, 3, (CHa, CHb), cnts, (idxA_in, idxB_in),
                        (dstA_in, dstB_in), None,
                        hcat3_full, R2, 1, ad3T_d,
                        iota, ident, b3r, h3T_d)

            # ---- final linear ----
            if lvl >= 10:
             with tc.tile_pool(name="fin", bufs=3) as mp, \
                 tc.tile_pool(name="finp", bufs=2, space="PSUM") as mpp:
                for t in range(NT):
                    lt = mp.tile([HID, P], bf16, tag="lt")
                    nc.sync.dma_start(out=lt[:],
                                      in_=h3T_d[:, t * P:(t + 1) * P])
                    o_ps = mpp.tile([P, D_OUT], f32, space="PSUM", tag="o")
                    nc.tensor.matmul(o_ps[:], lhsT=lt[:], rhs=WcT[:],
                                     start=True, stop=True)
                    ob = mp.tile([P, D_OUT], f32, tag="ob")
                    nc.vector.tensor_tensor(out=ob[:], in0=o_ps[:],
                                            in1=bcr[:], op=AT.add)
                    r = rows_of(t)
                    nc.sync.dma_start(out=out_d[t * P:t * P + r, :],
                                      in_=ob[:r, :])

    nc.compile()
    return nc


def prepare(**inputs):
    """Host preprocessing + program build; returns (nc, in_maps)."""
    x = np.asarray(inputs["x"], np.float32)
    edge_index = np.asarray(inputs["edge_index"])
    W1 = np.asarray(inputs["W1"], np.float32)
    a1_src = np.asarray(inputs["a1_src"], np.float32)
    a1_dst = np.asarray(inputs["a1_dst"], np.float32)
    b1 = np.asarray(inputs["b1"], np.float32)
    W2 = np.asarray(inputs["W2"], np.float32)
    a2_src = np.asarray(inputs["a2_src"], np.float32)
    a2_dst = np.asarray(inputs["a2_dst"], np.float32)
    b2 = np.asarray(inputs["b2"], np.float32)
    W3 = np.asarray(inputs["W3"], np.float32)
    a3_src = np.asarray(inputs["a3_src"], np.float32)
    a3_dst = np.asarray(inputs["a3_dst"], np.float32)
    b3 = np.asarray(inputs["b3"], np.float32)
    Wc = np.asarray(inputs["Wc"], np.float32)
    bc = np.asarray(inputs["bc"], np.float32)

    (CHa, CHb), cnts, idx_arrs, dst_arrs = _prep_edges(edge_index)

    # weight preprocessing
    W1h = W1.reshape(HEADS, HID, D_IN)
    M1s = np.einsum("hci,hc->ih", W1h, a1_src).astype(np.float32)  # [128, 8]
    M1d = np.einsum("hci,hc->ih", W1h, a1_dst).astype(np.float32)
    a_s1 = x @ M1s                                                # [N, 8]
    a_d1 = x @ M1d                                                # [N, 8]

    xTf = np.zeros((P, NFP), bfnp)
    xTf[:, :N] = x.T.astype(bfnp)

    common = {
        "xTf": xTf,
        "W1T": np.ascontiguousarray(W1.T).astype(bfnp),
        "W2T": np.ascontiguousarray(W2.T).astype(bfnp),
        "M2d": (W2.T @ a2_dst[0]).reshape(512, 1).astype(bfnp),
        "M2s": (W2.T @ a2_src[0]).reshape(512, 1).astype(bfnp),
        "W3T": np.ascontiguousarray(W3.T).astype(bfnp),
        "M3d": (W3.T @ a3_dst[0]).reshape(HID, 1).astype(bfnp),
        "M3s": (W3.T @ a3_src[0]).reshape(HID, 1).astype(bfnp),
        "WcT": np.ascontiguousarray(Wc.T).astype(bfnp),
        "b1r": np.tile(b1, (P, 1)),
        "b2r": np.tile(b2, (P, 1)),
        "b3r": np.tile(b3, (P, 1)),
        "bcr": np.tile(bc, (P, 1)),
    }

    in_maps = []
    tbase = (np.arange(NT) * P)[:, None, None]
    for k in range(NCORES):
        m = dict(common)
        m["idxA"] = idx_arrs[0][k]
        m["idxB"] = idx_arrs[1][k]
        m["dstA"] = dst_arrs[0][k]
        m["dstB"] = dst_arrs[1][k]
        for g, nm in ((0, "adEA"), (1, "adEB")):
            Dm = dst_arrs[g][k]                      # [NT, P, CH], -1 pad
            nodes = (k * VP + tbase + Dm).astype(np.int64)
            vals = a_d1[nodes.clip(0, N - 1)]        # [NT, P, CH, 8]
            ch = Dm.shape[2]
            # src index per slot from the wrapped idx array (idx k == ii)
            srcw = idx_arrs[g][k][:, 0:16, :].reshape(NT, 16, ch, 8)
            srcw = srcw.transpose(0, 3, 1, 2).reshape(NT, P, ch)
            svals = a_s1[(srcw.astype(np.int64) + g * OV0).clip(0, N - 1)]
            vals = vals + svals
            vals[Dm < 0] = 0.0
            m[nm] = np.ascontiguousarray(
                vals.reshape(NT, P, -1)).astype(bfnp)
        in_maps.append(m)

    nc = _build_program(CHa, CHb, cnts)
    return nc, in_maps


def kernel(**inputs):
    nc, in_maps = prepare(**inputs)
    r = run_bass_kernel_spmd(nc, in_maps, core_ids=list(range(NCORES)))
    out = np.concatenate([r.results[k]["out"][:VP] for k in range(NCORES)], 0)
    return out.astype(np.float32)
